# revision 16
# baseline (speedup 1.0000x reference)
"""Equivariant block-diagonal linear (irreps 256x0e + 256x1o + 128x2e) on 8
Trainium2 NeuronCores.

Math: for each irrep segment (mul, ird), out[b, v, i] = c * sum_u w[u,v] *
x[b, u, i] with c = 1/sqrt(mul). x columns are laid out mul-major:
col = seg_off + u*ird + i.

Data-parallel sharding: batch 50000 -> 8 cores x 6272 rows (padded).
Per core, per 128-row batch tile:
  1. DMA x rows to SBUF (256-row superloads for DMA efficiency).
  2. For each (segment, i, u-chunk): PE-transpose the strided column slice
     x[:, off+i::ird] (128 u values) into a shared PSUM tile (two
     transposes per tile) -> one DVE cast [128,256] to SBUF f32r.
  3. matmul(psum[b, v] += xT[u_chunk].T @ w[u_chunk, :]): seg1/seg2 use
     f32r (1 cycle/row at N=256), seg3 uses fp32 (N=128 would fall off
     the f32r fast path). Per segment the per-i matmuls write disjoint
     slices of one PSUM region, each slice within a single PSUM bank.
  4. One DVE copy per segment de-interleaves PSUM -> the [128, 1664]
     output staging tile (strided dst). One DMA per 256 rows to DRAM.

Weights are host-prescaled by c and stay SBUF-resident in natural [u, v]
layout (f32r copies for seg1/2).
"""

import numpy as np

N_CORES = 8
BATCH = 50000
X_DIM = 1664
P = 128
ROWS_PER_CORE = 6272  # 49 tiles of 128; 8*6272 = 50176 >= 50000
BT = ROWS_PER_CORE // P

# (mul, ird, x/y col offset, n u-chunks of 128)
SEGS = [
    (256, 1, 0, 2),
    (256, 3, 256, 2),
    (128, 5, 1024, 1),
]

_cache = {}


def _build_program(mode: str):
    import concourse.bacc as bacc
    import concourse.mybir as mybir
    from concourse.tile import TileContext

    use_f32r = mode == "float32r"
    f32 = mybir.dt.float32
    f32r = mybir.dt.float32r
    mm_dt = f32r if use_f32r else f32

    nc = bacc.Bacc(
        "TRN2", target_bir_lowering=False, debug=False, num_devices=N_CORES
    )
    x = nc.dram_tensor("x", [ROWS_PER_CORE, X_DIM], f32, kind="ExternalInput")
    w1 = nc.dram_tensor("w1", [256, 256], f32, kind="ExternalInput")
    w2 = nc.dram_tensor("w2", [256, 256], f32, kind="ExternalInput")
    w3 = nc.dram_tensor("w3", [128, 128], f32, kind="ExternalInput")
    ident = nc.dram_tensor("ident", [P, P], f32, kind="ExternalInput")
    y = nc.dram_tensor("y", [ROWS_PER_CORE, X_DIM], f32, kind="ExternalOutput")
    w_dram = [w1, w2, w3]
    # seg3 runs fp32 (N=128 misses the f32r >=256 fast path anyway).
    seg_dt = [mm_dt, mm_dt, f32]

    with TileContext(nc) as tc:
        with (
            tc.tile_pool(name="wpool", bufs=1) as wpool,
            tc.tile_pool(name="xin", bufs=4) as xin,
            tc.tile_pool(name="xtp", bufs=3, space="PSUM") as xtp,
            tc.tile_pool(name="xts", bufs=10) as xts,
            tc.tile_pool(name="po1p", bufs=1, space="PSUM") as po1p,
            tc.tile_pool(name="po2p", bufs=1, space="PSUM") as po2p,
            tc.tile_pool(name="po3p", bufs=1, space="PSUM") as po3p,
            tc.tile_pool(name="outp", bufs=4) as outp,
        ):
            # Resident weights, natural [u, v] layout, one [128, mul] chunk
            # per 128 u's. f32r inputs must be produced rounded, hence the
            # staged DVE copy.
            w_sb = []
            for si, (mul, ird, off, n_uc) in enumerate(SEGS):
                chunks = []
                for uc in range(n_uc):
                    # gpsimd (SWDGE) keeps these off the SP/ACT HWDGE rings
                    # so the first x loads start immediately.
                    t = wpool.tile([P, mul], seg_dt[si], tag=f"w{si}_{uc}")
                    if seg_dt[si] == f32:
                        nc.gpsimd.dma_start(
                            out=t[:], in_=w_dram[si][uc * P:(uc + 1) * P, :]
                        )
                    else:
                        stg = wpool.tile([P, mul], f32, tag=f"wstg{si}_{uc}")
                        nc.gpsimd.dma_start(
                            out=stg[:], in_=w_dram[si][uc * P:(uc + 1) * P, :]
                        )
                        nc.vector.tensor_copy(out=t[:], in_=stg[:])
                    chunks.append(t)
                w_sb.append(chunks)
            ident_sb = wpool.tile([P, P], f32, tag="ident")
            nc.gpsimd.dma_start(out=ident_sb[:], in_=ident[:, :])

            # Batch loop: pairs of 128-row tiles share one DMA (1.7 MB
            # transfers), with a single-tile epilogue if BT is odd.
            groups = [(g * 2, 2) for g in range(BT // 2)]
            if BT % 2:
                groups.append((BT - 1, 1))

            for bt0, ntile in groups:
                r0 = bt0 * P
                xt = xin.tile([P, 2 * X_DIM], f32, tag="x")
                nc.sync.dma_start(
                    out=xt[:, :ntile * X_DIM].rearrange(
                        "p (t c) -> p t c", t=ntile
                    ),
                    in_=x[r0:r0 + ntile * P, :].rearrange(
                        "(t p) c -> p t c", p=P
                    ),
                )
                ot = outp.tile([P, 2 * X_DIM], f32, tag="o")

                for t in range(ntile):
                    xoff = t * X_DIM

                    # --- transposes + casts: (seg, i, uc) -> xs tiles ---
                    # xs_map[(si, i, uc)] = (tile, col0)
                    xs_map = {}
                    pend = []  # pending halves in current tp/xs pair

                    def flush(pend):
                        if not pend:
                            return
                        width = P * len(pend)
                        dt_ = pend[0][3]
                        tp = xtp.tile([P, 2 * P], f32, tag="tp")
                        for h, (si, i, uc, _d, src) in enumerate(pend):
                            nc.tensor.transpose(
                                tp[:, h * P:(h + 1) * P], src, ident_sb[:]
                            )
                        xs = xts.tile([P, 2 * P], dt_, tag="xs")
                        nc.vector.tensor_copy(
                            out=xs[:, :width], in_=tp[:, :width]
                        )
                        for h, (si, i, uc, _d, src) in enumerate(pend):
                            xs_map[(si, i, uc)] = (xs, h * P)
                        pend.clear()

                    for si, (mul, ird, off, n_uc) in enumerate(SEGS):
                        for i in range(ird):
                            for uc in range(n_uc):
                                start = xoff + off + uc * P * ird + i
                                src = xt[:, start:start + ird * (P - 1) + 1:ird]
                                if pend and pend[0][3] != seg_dt[si]:
                                    flush(pend)
                                pend.append((si, i, uc, seg_dt[si], src))
                                if len(pend) == 2:
                                    flush(pend)
                    flush(pend)

                    # --- matmuls into per-segment PSUM regions ---
                    po1 = po1p.tile([P, 256], f32, tag="po1")
                    po2 = po2p.tile([P, 768], f32, tag="po2")
                    po3 = po3p.tile([P, 640], f32, tag="po3")
                    pos = [po1, po2, po3]
                    for si, (mul, ird, off, n_uc) in enumerate(SEGS):
                        for i in range(ird):
                            dst = pos[si][:, i * mul:(i + 1) * mul]
                            for uc in range(n_uc):
                                xs, c0 = xs_map[(si, i, uc)]
                                nc.tensor.matmul(
                                    dst,
                                    xs[:, c0:c0 + P],
                                    w_sb[si][uc][:],
                                    start=(uc == 0),
                                    stop=(uc == n_uc - 1),
                                )

                    # --- de-interleave PSUM -> output staging ---
                    # Strided-dst copies are slow; the scalar engine is idle,
                    # so it takes the two interleaved segments while DVE
                    # keeps the contiguous seg1 copy (plus casts above).
                    for si, (mul, ird, off, n_uc) in enumerate(SEGS):
                        seg_w = mul * ird
                        src = pos[si][:].rearrange("p (i v) -> p i v", i=ird)
                        dst = ot[
                            :, xoff + off:xoff + off + seg_w
                        ].rearrange("p (v i) -> p i v", i=ird)
                        if si == 0:
                            nc.vector.tensor_copy(out=dst, in_=src)
                        else:
                            nc.scalar.copy(out=dst, in_=src)

                # Output stores ride the ACT HWDGE ring so input prefetch
                # (SP ring) never queues behind them.
                nc.scalar.dma_start(
                    out=y[r0:r0 + ntile * P, :].rearrange(
                        "(t p) c -> p t c", p=P
                    ),
                    in_=ot[:, :ntile * X_DIM].rearrange(
                        "p (t c) -> p t c", t=ntile
                    ),
                )

    nc.compile()
    return nc


def _get_program(mode: str):
    if mode not in _cache:
        _cache[mode] = _build_program(mode)
    return _cache[mode]


def kernel(x: np.ndarray, weight: np.ndarray, _mm_dtype: str = "float32r",
           _trace: bool = False):
    from concourse.bass_utils import run_bass_kernel_spmd

    nc = _get_program(_mm_dtype)

    x = np.ascontiguousarray(np.asarray(x, dtype=np.float32))
    weight = np.asarray(weight, dtype=np.float32)

    w1 = (weight[:65536].reshape(256, 256) / np.sqrt(np.float32(256.0))).astype(np.float32)
    w2 = (weight[65536:131072].reshape(256, 256) / np.sqrt(np.float32(256.0))).astype(np.float32)
    w3 = (weight[131072:].reshape(128, 128) / np.sqrt(np.float32(128.0))).astype(np.float32)
    ident = np.eye(P, dtype=np.float32)

    xp = np.zeros((N_CORES * ROWS_PER_CORE, X_DIM), dtype=np.float32)
    xp[:BATCH] = x

    in_maps = [
        {
            "x": xp[c * ROWS_PER_CORE:(c + 1) * ROWS_PER_CORE],
            "w1": w1,
            "w2": w2,
            "w3": w3,
            "ident": ident,
        }
        for c in range(N_CORES)
    ]
    res = run_bass_kernel_spmd(
        nc, in_maps, list(range(N_CORES)), trace=_trace
    )
    out = np.concatenate([res.results[c]["y"] for c in range(N_CORES)], axis=0)
    if _trace:
        kernel.last_exec_time_ns = res.exec_time_ns
    return out[:BATCH]


# revision 17
# speedup vs baseline: 1.0191x; 1.0191x over previous
"""Equivariant block-diagonal linear (irreps 256x0e + 256x1o + 128x2e) on 8
Trainium2 NeuronCores.

Math: for each irrep segment (mul, ird), out[b, v, i] = c * sum_u w[u,v] *
x[b, u, i] with c = 1/sqrt(mul). x columns are laid out mul-major:
col = seg_off + u*ird + i.

Data-parallel sharding: batch 50000 -> 8 cores x 6272 rows (padded).
Per core, per 128-row batch tile:
  1. DMA x rows to SBUF (256-row superloads for DMA efficiency).
  2. For each (segment, i, u-chunk): PE-transpose the strided column slice
     x[:, off+i::ird] (128 u values) into a shared PSUM tile (two
     transposes per tile) -> one DVE cast [128,256] to SBUF f32r.
  3. matmul(psum[b, v] += xT[u_chunk].T @ w[u_chunk, :]): seg1/seg2 use
     f32r (1 cycle/row at N=256), seg3 uses fp32 (N=128 would fall off
     the f32r fast path). Per segment the per-i matmuls write disjoint
     slices of one PSUM region, each slice within a single PSUM bank.
  4. One DVE copy per segment de-interleaves PSUM -> the [128, 1664]
     output staging tile (strided dst). One DMA per 256 rows to DRAM.

Weights are host-prescaled by c and stay SBUF-resident in natural [u, v]
layout (f32r copies for seg1/2).
"""

import numpy as np

N_CORES = 8
BATCH = 50000
X_DIM = 1664
P = 128
ROWS_PER_CORE = 6272  # 49 tiles of 128; 8*6272 = 50176 >= 50000
BT = ROWS_PER_CORE // P

# (mul, ird, x/y col offset, n u-chunks of 128)
SEGS = [
    (256, 1, 0, 2),
    (256, 3, 256, 2),
    (128, 5, 1024, 1),
]

_cache = {}


def _build_program(mode: str):
    import concourse.bacc as bacc
    import concourse.mybir as mybir
    from concourse.tile import TileContext

    use_f32r = mode == "float32r"
    f32 = mybir.dt.float32
    f32r = mybir.dt.float32r
    mm_dt = f32r if use_f32r else f32

    nc = bacc.Bacc(
        "TRN2", target_bir_lowering=False, debug=False, num_devices=N_CORES
    )
    x = nc.dram_tensor("x", [ROWS_PER_CORE, X_DIM], f32, kind="ExternalInput")
    w1 = nc.dram_tensor("w1", [256, 256], f32, kind="ExternalInput")
    w2 = nc.dram_tensor("w2", [256, 256], f32, kind="ExternalInput")
    w3 = nc.dram_tensor("w3", [128, 128], f32, kind="ExternalInput")
    ident = nc.dram_tensor("ident", [P, P], f32, kind="ExternalInput")
    y = nc.dram_tensor("y", [ROWS_PER_CORE, X_DIM], f32, kind="ExternalOutput")
    w_dram = [w1, w2, w3]
    # seg3 runs fp32 (N=128 misses the f32r >=256 fast path anyway).
    seg_dt = [mm_dt, mm_dt, f32]

    with TileContext(nc) as tc:
        with (
            tc.tile_pool(name="wpool", bufs=1) as wpool,
            tc.tile_pool(name="xin", bufs=4) as xin,
            tc.tile_pool(name="xtp", bufs=3, space="PSUM") as xtp,
            tc.tile_pool(name="xts", bufs=10) as xts,
            tc.tile_pool(name="po1p", bufs=1, space="PSUM") as po1p,
            tc.tile_pool(name="po2p", bufs=1, space="PSUM") as po2p,
            tc.tile_pool(name="po3p", bufs=1, space="PSUM") as po3p,
            tc.tile_pool(name="outp", bufs=4) as outp,
        ):
            # Resident weights, natural [u, v] layout, one [128, mul] chunk
            # per 128 u's. f32r inputs must be produced rounded, hence the
            # staged DVE copy.
            w_sb = []
            for si, (mul, ird, off, n_uc) in enumerate(SEGS):
                chunks = []
                for uc in range(n_uc):
                    t = wpool.tile([P, mul], seg_dt[si], tag=f"w{si}_{uc}")
                    if seg_dt[si] == f32:
                        nc.sync.dma_start(
                            out=t[:], in_=w_dram[si][uc * P:(uc + 1) * P, :]
                        )
                    else:
                        stg = wpool.tile([P, mul], f32, tag=f"wstg{si}_{uc}")
                        nc.sync.dma_start(
                            out=stg[:], in_=w_dram[si][uc * P:(uc + 1) * P, :]
                        )
                        nc.vector.tensor_copy(out=t[:], in_=stg[:])
                    chunks.append(t)
                w_sb.append(chunks)
            ident_sb = wpool.tile([P, P], f32, tag="ident")
            nc.sync.dma_start(out=ident_sb[:], in_=ident[:, :])

            # Batch loop: pairs of 128-row tiles share one DMA (1.7 MB
            # transfers), with a single-tile epilogue if BT is odd.
            groups = [(g * 2, 2) for g in range(BT // 2)]
            if BT % 2:
                groups.append((BT - 1, 1))

            for bt0, ntile in groups:
                r0 = bt0 * P
                xt = xin.tile([P, 2 * X_DIM], f32, tag="x")
                nc.sync.dma_start(
                    out=xt[:, :ntile * X_DIM].rearrange(
                        "p (t c) -> p t c", t=ntile
                    ),
                    in_=x[r0:r0 + ntile * P, :].rearrange(
                        "(t p) c -> p t c", p=P
                    ),
                )
                ot = outp.tile([P, 2 * X_DIM], f32, tag="o")

                for t in range(ntile):
                    xoff = t * X_DIM

                    # --- transposes + casts: (seg, i, uc) -> xs tiles ---
                    # xs_map[(si, i, uc)] = (tile, col0)
                    xs_map = {}
                    pend = []  # pending halves in current tp/xs pair

                    def flush(pend):
                        if not pend:
                            return
                        width = P * len(pend)
                        dt_ = pend[0][3]
                        tp = xtp.tile([P, 2 * P], f32, tag="tp")
                        for h, (si, i, uc, _d, src) in enumerate(pend):
                            nc.tensor.transpose(
                                tp[:, h * P:(h + 1) * P], src, ident_sb[:]
                            )
                        xs = xts.tile([P, 2 * P], dt_, tag="xs")
                        nc.vector.tensor_copy(
                            out=xs[:, :width], in_=tp[:, :width]
                        )
                        for h, (si, i, uc, _d, src) in enumerate(pend):
                            xs_map[(si, i, uc)] = (xs, h * P)
                        pend.clear()

                    for si, (mul, ird, off, n_uc) in enumerate(SEGS):
                        for i in range(ird):
                            for uc in range(n_uc):
                                start = xoff + off + uc * P * ird + i
                                src = xt[:, start:start + ird * (P - 1) + 1:ird]
                                if pend and pend[0][3] != seg_dt[si]:
                                    flush(pend)
                                pend.append((si, i, uc, seg_dt[si], src))
                                if len(pend) == 2:
                                    flush(pend)
                    flush(pend)

                    # --- matmuls into per-segment PSUM regions ---
                    po1 = po1p.tile([P, 256], f32, tag="po1")
                    po2 = po2p.tile([P, 768], f32, tag="po2")
                    po3 = po3p.tile([P, 640], f32, tag="po3")
                    pos = [po1, po2, po3]
                    for si, (mul, ird, off, n_uc) in enumerate(SEGS):
                        for i in range(ird):
                            dst = pos[si][:, i * mul:(i + 1) * mul]
                            for uc in range(n_uc):
                                xs, c0 = xs_map[(si, i, uc)]
                                nc.tensor.matmul(
                                    dst,
                                    xs[:, c0:c0 + P],
                                    w_sb[si][uc][:],
                                    start=(uc == 0),
                                    stop=(uc == n_uc - 1),
                                )

                    # --- de-interleave PSUM -> output staging ---
                    # Strided-dst copies are slow; the scalar engine is idle,
                    # so it takes the two interleaved segments while DVE
                    # keeps the contiguous seg1 copy (plus casts above).
                    for si, (mul, ird, off, n_uc) in enumerate(SEGS):
                        seg_w = mul * ird
                        src = pos[si][:].rearrange("p (i v) -> p i v", i=ird)
                        dst = ot[
                            :, xoff + off:xoff + off + seg_w
                        ].rearrange("p (v i) -> p i v", i=ird)
                        if si == 0:
                            nc.vector.tensor_copy(out=dst, in_=src)
                        else:
                            nc.scalar.copy(out=dst, in_=src)

                # Output stores ride the ACT HWDGE ring so input prefetch
                # (SP ring) never queues behind them.
                nc.scalar.dma_start(
                    out=y[r0:r0 + ntile * P, :].rearrange(
                        "(t p) c -> p t c", p=P
                    ),
                    in_=ot[:, :ntile * X_DIM].rearrange(
                        "p (t c) -> p t c", t=ntile
                    ),
                )

    nc.compile()
    return nc


def _get_program(mode: str):
    if mode not in _cache:
        _cache[mode] = _build_program(mode)
    return _cache[mode]


def kernel(x: np.ndarray, weight: np.ndarray, _mm_dtype: str = "float32r",
           _trace: bool = False):
    from concourse.bass_utils import run_bass_kernel_spmd

    nc = _get_program(_mm_dtype)

    x = np.ascontiguousarray(np.asarray(x, dtype=np.float32))
    weight = np.asarray(weight, dtype=np.float32)

    w1 = (weight[:65536].reshape(256, 256) / np.sqrt(np.float32(256.0))).astype(np.float32)
    w2 = (weight[65536:131072].reshape(256, 256) / np.sqrt(np.float32(256.0))).astype(np.float32)
    w3 = (weight[131072:].reshape(128, 128) / np.sqrt(np.float32(128.0))).astype(np.float32)
    ident = np.eye(P, dtype=np.float32)

    xp = np.zeros((N_CORES * ROWS_PER_CORE, X_DIM), dtype=np.float32)
    xp[:BATCH] = x

    in_maps = [
        {
            "x": xp[c * ROWS_PER_CORE:(c + 1) * ROWS_PER_CORE],
            "w1": w1,
            "w2": w2,
            "w3": w3,
            "ident": ident,
        }
        for c in range(N_CORES)
    ]
    res = run_bass_kernel_spmd(
        nc, in_maps, list(range(N_CORES)), trace=_trace
    )
    out = np.concatenate([res.results[c]["y"] for c in range(N_CORES)], axis=0)
    if _trace:
        kernel.last_exec_time_ns = res.exec_time_ns
    return out[:BATCH]
